# revision 14
# baseline (speedup 1.0000x reference)
"""Trainium2 Bass kernel for nn_BatchConv1d (dynamic grouped conv attention).

Reference computation (per batch b):
    kernel = (q @ W_kernel + b_kernel).reshape(Lq, C, KW)      # dynamic conv kernels
    bias   = (q @ W_bias + b_bias)[:, 0]
    kpad   = zero-pad k along L by PAD=1
    a[i,j] = sum_{c,w} kernel[i,c,w] * kpad[j+w,c] + bias[i] + bias_b

Strategy: data-parallel over B=8 (one batch per NeuronCore). Per core:
  Inputs ship from host as bf16, pre-transposed (qT[d, i], padded kT[c, x])
  and host-permuted (W), so plain contiguous DMAs deliver matmul-ready
  layouts -- the PE does zero transposes and runs only bf16 matmuls:
  Stage 1 (PE): kernelT[cw, i] = sum_d Wp[d, cw] * qT[d, i], with Wp host-
     permuted so cw = w*C + c and an extra column 1536 holding W_bias whose
     output row is the per-query bias. b_kernel is folded in during the
     PSUM->SBUF copy (DVE, cast to bf16); the bias row gets b_bias + bias_b.
  Stage 2 (PE): out[i, j] = sum_{ct,w} kernelT[w*4+ct][:, i] . kT_pad[ct][:, j+w]
     kT_pad (k shipped host-padded: zero row each side of the XBAR transpose)
     makes the 3 window shifts free-dim offsets; the per-query bias (column form via 8 tiny K=1 matmuls) is added
     during the output PSUM->SBUF copy (Activation engine), whose queue also
     carries the output DMA so the write dispatch needs no extra sync.
  All input tiles double-buffer across repeats (ping/pong by rep parity), so
  the next rep's DMAs dispatch and transfer entirely under the current rep's
  compute and the PE rolls across rep boundaries without idling.  Input DMAs
  ride the SP HWDGE queue; output DMAs ride the Activation HWDGE queue.
"""

import numpy as np
import ml_dtypes
from contextlib import ExitStack

import concourse.bass as bass
import concourse.mybir as mybir
import concourse.tile as tile
from concourse import bacc
from concourse.bass_utils import run_bass_kernel_spmd

F32 = mybir.dt.float32
BF16 = mybir.dt.bfloat16
ACT_IDENT = mybir.ActivationFunctionType.Identity

B, Lq, Lk, D, C, KW = 8, 1024, 1024, 512, 512, 3
CW = C * KW            # 1536
CWE = CW + 1           # 1537 (last col = W_bias)
NT_I = Lq // 128       # 8 i-tiles
NT_D = D // 128        # 4 d-tiles
NT_CW = CW // 128      # 12 kernel col tiles
NJ = 2                 # j chunks of 512
LKP = 1280             # kT row length: 1026 used cols, padded so rows are 512B-aligned

_CACHE = {}

import os
S1BUFS = int(os.environ.get("S1BUFS", "4"))
S2BUFS = int(os.environ.get("S2BUFS", "3"))


def _build(repeats=1):
    nc = bacc.Bacc(target_bir_lowering=False, debug=False)

    q_in = nc.dram_tensor("q_in", [D, Lq], BF16, kind="ExternalInput").ap()
    k_in = nc.dram_tensor("k_in", [C, LKP], BF16, kind="ExternalInput").ap()
    wp_in = nc.dram_tensor("wp_in", [D, CWE], BF16, kind="ExternalInput").ap()
    # [128, 13]: cols 0..11 = b_kernel tiles, col 12 row 0 = b_bias + bias_b
    bkp_in = nc.dram_tensor("bkp_in", [128, NT_CW + 1], F32, kind="ExternalInput").ap()
    out = nc.dram_tensor("out", [Lq, Lk], F32, kind="ExternalOutput").ap()

    with tile.TileContext(nc) as tc, ExitStack() as ctx:
        persist = ctx.enter_context(tc.tile_pool(name="persist", bufs=1))
        out_pool = ctx.enter_context(tc.tile_pool(name="outp", bufs=3))
        s1psum = ctx.enter_context(tc.tile_pool(name="s1psum", bufs=S1BUFS, space="PSUM"))
        s2psum = ctx.enter_context(tc.tile_pool(name="s2psum", bufs=S2BUFS, space="PSUM"))

        st = {}
        st["qT"] = [persist.tile([128, NT_D * Lq], BF16, tag=f"qT{p}", name=f"qT{p}")
                    for p in range(2)]
        st["kT"] = [persist.tile([128, NT_D * LKP], BF16, tag=f"kT{p}", name=f"kT{p}")
                    for p in range(2)]
        st["wp"] = [persist.tile([128, NT_D * CWE], BF16, tag=f"wp{p}",
                                 name=f"wp{p}") for p in range(2)]
        st["bkp"] = [persist.tile([128, NT_CW + 1], F32, tag=f"bkp{p}",
                                  name=f"bkp{p}") for p in range(2)]
        st["kernelT"] = [persist.tile([128, Lq], BF16, tag=f"kern{t}", name=f"kern{t}")
                         for t in range(NT_CW)]
        st["bias_row"] = persist.tile([1, Lq], F32, tag="bias_row", name="bias_row")
        st["bias_col"] = persist.tile([128, NT_I], F32, tag="bias_col", name="bias_col")
        st["one_t"] = persist.tile([1, 1], F32, tag="one_t", name="one_t")
        nc.vector.memset(st["one_t"][:], 1.0)

        pools = (out_pool, s1psum, s2psum)
        for rep in range(repeats):
            _emit_rep(nc, rep, st, pools, q_in, k_in, wp_in, bkp_in, out)

    nc.compile()
    return nc


def _emit_rep(nc, rep, st, pools, q_in, k_in, wp_in, bkp_in, out):
    out_pool, s1psum, s2psum = pools
    p = rep % 2
    qT_all, kT_all = st["qT"][p], st["kT"][p]
    wp_all, bkp_col = st["wp"][p], st["bkp"][p]
    wp_sb = [wp_all[:, t * CWE:(t + 1) * CWE] for t in range(NT_D)]
    kernelT = st["kernelT"]
    bias_row, bias_col, one_t = st["bias_row"], st["bias_col"], st["one_t"]
    bconst = bkp_col[0:1, NT_CW:NT_CW + 1]

    qT = [qT_all[:, d * Lq:(d + 1) * Lq] for d in range(NT_D)]
    kT_pad = [kT_all[:, c * LKP:(c + 1) * LKP] for c in range(NT_D)]

    # ---- input DMAs (SP queue), in rep-0 consumption order ----------------
    nc.sync.dma_start(bkp_col[:], bkp_in[:])
    wp_dst = wp_all[:].rearrange("p (t c) -> p t c", t=NT_D)
    wp_src = wp_in.rearrange("(t p) c -> p t c", p=128)
    nc.sync.dma_start(wp_dst[:, :, 0:512], wp_src[:, :, 0:512])
    nc.sync.dma_start(qT_all[:].rearrange("p (t i) -> p t i", t=NT_D),
                      q_in.rearrange("(t p) i -> p t i", p=128))
    nc.sync.dma_start(wp_dst[:, :, 512:CWE], wp_src[:, :, 512:CWE])
    nc.sync.dma_start(kT_all[:].rearrange("p (t x) -> p t x", t=NT_D),
                      k_in.rearrange("(t p) x -> p t x", p=128))

    # ---- stage 1: kernelT[cw, i] (+ bias row via W_bias column) -----------
    def emit_s1(mts, njcs):
        for mt in mts:
            for njc in njcs:
                ps = s1psum.tile([128, 512], F32, tag="s1", name=f"r{rep}s1")
                if mt < NT_CW:
                    for dt in range(NT_D):
                        nc.tensor.matmul(
                            ps[:],
                            wp_sb[dt][:, mt * 128:(mt + 1) * 128],
                            qT[dt][:, njc * 512:(njc + 1) * 512],
                            start=(dt == 0),
                            stop=(dt == NT_D - 1),
                        )
                    nc.vector.tensor_scalar_add(
                        kernelT[mt][:, njc * 512:(njc + 1) * 512],
                        ps[:], bkp_col[:, mt:mt + 1]
                    )
                else:
                    for dt in range(NT_D):
                        nc.tensor.matmul(
                            ps[0:1, :],
                            wp_sb[dt][:, CW:CWE],
                            qT[dt][:, njc * 512:(njc + 1) * 512],
                            start=(dt == 0),
                            stop=(dt == NT_D - 1),
                        )
                    nc.vector.tensor_scalar_add(
                        bias_row[:, njc * 512:(njc + 1) * 512],
                        ps[0:1, :], bconst
                    )

    emit_s1(range(0, NT_CW + 1), [0])

    # bias row half -> column form via 4 tiny K=1 matmuls
    bias_ps = s1psum.tile([128, NT_I], F32, tag="bias_ps", bufs=1)

    def emit_bias_cols(ts):
        for t in ts:
            nc.tensor.matmul(
                bias_ps[:, t:t + 1],
                bias_row[:, t * 128:(t + 1) * 128],
                one_t[:],
                start=True, stop=True,
            )
        nc.vector.tensor_copy(bias_col[:, ts[0]:ts[-1] + 1],
                              bias_ps[:, ts[0]:ts[-1] + 1])

    emit_bias_cols(range(0, 4))

    # ---- stage 2: out[i, j] conv matmuls; bias added in Activation copy ---
    def emit_s2(its):
        for it in its:
            o_sb = out_pool.tile([128, Lk], F32, tag="osb", name=f"r{rep}osb")
            for jc in range(NJ):
                ps = s2psum.tile([128, 512], F32, tag="s2", name=f"r{rep}s2")
                idx = 0
                for w in range(KW):
                    for ct in range(NT_D):
                        nc.tensor.matmul(
                            ps[:],
                            kernelT[w * NT_D + ct][:, it * 128:(it + 1) * 128],
                            kT_pad[ct][:, jc * 512 + w:jc * 512 + w + 512],
                            start=(idx == 0),
                            stop=(idx == KW * NT_D - 1),
                        )
                        idx += 1
                nc.scalar.activation(o_sb[:, jc * 512:(jc + 1) * 512], ps[:],
                                     ACT_IDENT, bias=bias_col[:, it:it + 1])
            nc.scalar.dma_start(out[it * 128:(it + 1) * 128, :], o_sb[:])

    emit_s2(range(0, 4))
    emit_s1(range(0, NT_CW + 1), [1])
    emit_bias_cols(range(4, NT_I))
    emit_s2(range(4, NT_I))


def _get_nc():
    if "nc" not in _CACHE:
        _CACHE["nc"] = _build()
    return _CACHE["nc"]


def _prepare_in_maps(q, k, W_kernel, b_kernel, W_bias, b_bias, bias_b):
    q = np.asarray(q, dtype=np.float32)
    k = np.asarray(k, dtype=np.float32)
    W_kernel = np.asarray(W_kernel, dtype=np.float32)
    b_kernel = np.asarray(b_kernel, dtype=np.float32)
    W_bias = np.asarray(W_bias, dtype=np.float32)
    b_bias = np.asarray(b_bias, dtype=np.float32)
    bias_b = np.asarray(bias_b, dtype=np.float32)

    # host-side permutation: Wp[:, w*C + c] = W_kernel[:, c*KW + w]; col 1536 = W_bias
    Wp = W_kernel.reshape(D, C, KW).transpose(0, 2, 1).reshape(D, CW)
    Wp_ext = np.concatenate([Wp, W_bias.reshape(D, 1)], axis=1)
    wp_bf = np.ascontiguousarray(Wp_ext.astype(ml_dtypes.bfloat16))
    bkp = b_kernel.reshape(C, KW).T.reshape(CW)
    bkp_col = np.zeros((128, NT_CW + 1), np.float32)
    bkp_col[:, :NT_CW] = bkp.reshape(NT_CW, 128).T
    bkp_col[0, NT_CW] = b_bias.reshape(-1)[0] + bias_b.reshape(-1)[0]

    # pre-transposed on host: qT [D, Lq]; kT [C, LKP] with zero pad cols at
    # x=0 and x>=1025 (the conv 'same' pad)
    q_bf = np.ascontiguousarray(q.transpose(0, 2, 1).astype(ml_dtypes.bfloat16))
    k_bf = np.zeros((B, C, LKP), ml_dtypes.bfloat16)
    k_bf[:, :, 1:1 + Lk] = k.transpose(0, 2, 1).astype(ml_dtypes.bfloat16)

    return [
        {
            "q_in": np.ascontiguousarray(q_bf[b]),
            "k_in": np.ascontiguousarray(k_bf[b]),
            "wp_in": wp_bf,
            "bkp_in": bkp_col,
        }
        for b in range(B)
    ]


def kernel(q, k, W_kernel, b_kernel, W_bias, b_bias, bias_b):
    in_maps = _prepare_in_maps(q, k, W_kernel, b_kernel, W_bias, b_bias, bias_b)
    res = run_bass_kernel_spmd(_get_nc(), in_maps, core_ids=list(range(B)))
    return np.stack([res.results[b]["out"] for b in range(B)], axis=0)


def kernel_profiled(q, k, W_kernel, b_kernel, W_bias, b_bias, bias_b, **kw):
    """Like kernel() but with NTFF tracing; returns (output, BassKernelResults)."""
    in_maps = _prepare_in_maps(q, k, W_kernel, b_kernel, W_bias, b_bias, bias_b)
    res = run_bass_kernel_spmd(
        _get_nc(), in_maps, core_ids=list(range(B)), trace=True, **kw
    )
    out = np.stack([res.results[b]["out"] for b in range(B)], axis=0)
    return out, res
